# revision 1
# baseline (speedup 1.0000x reference)
"""GCN message-passing layer on 8 Trainium2 NeuronCores.

out = segment_sum(x[src], dst, N) @ W.T + b

Strategy (per core, dst-sharded):
  - Host: greedily bin-pack dst nodes into 8 cores x 40 tiles x 128 lanes,
    balancing in-degree so every tile has a uniform chunk budget. Edges are
    grouped per (tile, src-half) into 128-edge chunks (int16 gather indices
    require splitting the 40000-row x table at row 32768).
  - Device: dma_gather (SWDGE -> 16 SDMA engines) pulls x[src] rows from HBM
    into [128 edges, 128 feat] SBUF chunk tiles; a one-hot matrix P (built on
    DVE via iota==dst_local) scatters each chunk into a PSUM accumulator
    h_T[i, d] via PE matmul; a second PE matmul applies W.T and the bias.
  - Host: inverse-permute the per-core outputs back to node order.
"""

import sys

import numpy as np

sys.path.insert(0, "/opt/trn_rl_repo")

N_NODES = 40000
N_EDGES = 640000
D = 128
P = 128
N_CORES = 8
TILES_PER_CORE = 40
N_BINS = N_CORES * TILES_PER_CORE  # 320 tiles of up to 128 nodes
SLOTS_PER_CORE = TILES_PER_CORE * P  # 5120
HALF = 32768  # int16 gather index limit
ST = 4  # tiles per dma_gather call (supertile)
PAD_DST = 1000.0  # dst_local sentinel for pad slots -> all-zero one-hot column

_PROGRAM_CACHE: dict = {}


def _pack_nodes(dst: np.ndarray, src: np.ndarray):
    """Greedy LPT pack of nodes into N_BINS bins (<=128 nodes each),
    balancing per-bin in-degree of half-A edges. Returns bin/lane maps."""
    import heapq

    degA = np.bincount(dst[src < HALF], minlength=N_NODES)
    degB = np.bincount(dst[src >= HALF], minlength=N_NODES)
    order = np.argsort(-(degA + degB), kind="stable")

    heap = [(0, b) for b in range(N_BINS)]
    heapq.heapify(heap)
    counts = np.zeros(N_BINS, dtype=np.int64)
    node_bin = np.empty(N_NODES, dtype=np.int32)
    node_lane = np.empty(N_NODES, dtype=np.int32)
    stash = []
    for n in order:
        while True:
            s, b = heapq.heappop(heap)
            if counts[b] < P:
                break
            stash.append(None)  # full bin: drop permanently
        node_bin[n] = b
        node_lane[n] = counts[b]
        counts[b] += 1
        heapq.heappush(heap, (s + int(degA[n]) + int(degB[n]), b))
    return node_bin, node_lane


def _wrap_idx(arr: np.ndarray) -> np.ndarray:
    """int16 flat idx list -> [128, len/16] wrapped + replicated layout."""
    w = arr.reshape(-1, 16).T  # [16, n/16]
    return np.ascontiguousarray(np.tile(w, (8, 1)))


def _prepare(x, src, dst, W, b):
    src = np.asarray(src).astype(np.int64)
    dst = np.asarray(dst).astype(np.int64)
    node_bin, node_lane = _pack_nodes(dst, src)

    ebin = node_bin[dst]
    ehalf = (src >= HALF).astype(np.int64)
    edl = node_lane[dst].astype(np.float32)
    eorder = np.lexsort((src, ehalf, ebin))
    s_src, s_half, s_bin, s_dl = (
        src[eorder],
        ehalf[eorder],
        ebin[eorder],
        edl[eorder],
    )

    # per (bin, half) segment boundaries
    key = s_bin * 2 + s_half
    seg_start = np.searchsorted(key, np.arange(N_BINS * 2), side="left")
    seg_end = np.searchsorted(key, np.arange(N_BINS * 2), side="right")
    cntA = (seg_end[0::2] - seg_start[0::2]).reshape(N_CORES, TILES_PER_CORE)
    cntB = (seg_end[1::2] - seg_start[1::2]).reshape(N_CORES, TILES_PER_CORE)
    cap_a = int(np.ceil(cntA.max() / P))
    cap_b = max(1, int(np.ceil(cntB.max() / P)))
    cap = cap_a + cap_b

    # per-core tables
    idxA = np.zeros((N_CORES, TILES_PER_CORE, cap_a * P), dtype=np.int16)
    idxB = np.zeros((N_CORES, TILES_PER_CORE, cap_b * P), dtype=np.int16)
    dstloc = np.full((N_CORES, TILES_PER_CORE, cap, P), PAD_DST, dtype=np.float32)
    for c in range(N_CORES):
        for t in range(TILES_PER_CORE):
            g = (c * TILES_PER_CORE + t) * 2
            a0, a1 = seg_start[g], seg_end[g]
            nA = a1 - a0
            idxA[c, t, :nA] = s_src[a0:a1].astype(np.int16)
            flat = dstloc[c, t].reshape(-1)  # [cap*P] view, A then B slots
            flat[:nA] = s_dl[a0:a1]
            b0, b1 = seg_start[g + 1], seg_end[g + 1]
            nB = b1 - b0
            idxB[c, t, :nB] = (s_src[b0:b1] - HALF).astype(np.int16)
            flat[cap_a * P : cap_a * P + nB] = s_dl[b0:b1]

    # dstloc kernel layout: [128 lanes, TILES*cap chunks] per core
    dstloc_k = np.ascontiguousarray(
        dstloc.reshape(N_CORES, TILES_PER_CORE * cap, P).transpose(0, 2, 1)
    )

    iota = np.tile(np.arange(P, dtype=np.float32)[None, :], (P, 1))
    wt = np.ascontiguousarray(np.asarray(W).T.astype(np.float32))
    brow = np.asarray(b).astype(np.float32)[None, :]
    x = np.ascontiguousarray(np.asarray(x).astype(np.float32))

    in_maps = []
    for c in range(N_CORES):
        in_maps.append(
            {
                "x": x[:HALF],
                "xb": np.ascontiguousarray(x[HALF:]),
                "idxa": _wrap_idx(idxA[c].reshape(-1)),
                "idxb": _wrap_idx(idxB[c].reshape(-1)),
                "dstloc": dstloc_k[c],
                "wt": wt,
                "iota": iota,
                "brow": brow,
            }
        )

    # slot -> node map for output unpermute
    slot_node = np.full(N_BINS * P, -1, dtype=np.int64)
    slot_node[node_bin.astype(np.int64) * P + node_lane] = np.arange(N_NODES)
    return in_maps, (cap_a, cap_b), slot_node


def _build_program(cap_a: int, cap_b: int):
    import concourse.mybir as mybir
    import concourse.tile as tile
    from concourse import bacc

    cap = cap_a + cap_b
    f32 = mybir.dt.float32

    nc = bacc.Bacc("TRN2")
    x = nc.dram_tensor("x", [HALF, D], f32, kind="ExternalInput")
    xb = nc.dram_tensor("xb", [N_NODES - HALF, D], f32, kind="ExternalInput")
    idxa = nc.dram_tensor(
        "idxa", [P, TILES_PER_CORE * cap_a * P // 16], mybir.dt.int16,
        kind="ExternalInput",
    )
    idxb = nc.dram_tensor(
        "idxb", [P, TILES_PER_CORE * cap_b * P // 16], mybir.dt.int16,
        kind="ExternalInput",
    )
    dstloc = nc.dram_tensor(
        "dstloc", [P, TILES_PER_CORE * cap], f32, kind="ExternalInput"
    )
    wt = nc.dram_tensor("wt", [D, D], f32, kind="ExternalInput")
    iota_in = nc.dram_tensor("iota", [P, P], f32, kind="ExternalInput")
    brow = nc.dram_tensor("brow", [1, D], f32, kind="ExternalInput")
    out = nc.dram_tensor("out", [SLOTS_PER_CORE, D], f32, kind="ExternalOutput")

    G = int(__import__("os").environ.get("GCHUNKS", "8"))  # chunks per gather call
    cap_a_p = ((cap_a + G - 1) // G) * G  # padded to call granularity
    cap_b_p = ((cap_b + G - 1) // G) * G

    with tile.TileContext(nc) as tc:
        with (
            tc.tile_pool(name="const", bufs=1) as cpool,
            tc.tile_pool(name="ma", bufs=2) as ma_pool,
            tc.tile_pool(name="mb", bufs=2) as mb_pool,
            tc.tile_pool(name="pt", bufs=4) as p_pool,
            tc.tile_pool(name="ht", bufs=3) as ht_pool,
            tc.tile_pool(name="ot", bufs=3) as o_pool,
            tc.tile_pool(name="ps1", bufs=4, space="PSUM") as ps1_pool,
            tc.tile_pool(name="ps2", bufs=2, space="PSUM") as ps2_pool,
        ):
            idxa_t = cpool.tile([P, TILES_PER_CORE * cap_a * P // 16], mybir.dt.int16)
            nc.sync.dma_start(out=idxa_t[:], in_=idxa[:])
            idxb_t = cpool.tile([P, TILES_PER_CORE * cap_b * P // 16], mybir.dt.int16)
            nc.sync.dma_start(out=idxb_t[:], in_=idxb[:])
            dl_t = cpool.tile([P, TILES_PER_CORE * cap], f32)
            nc.sync.dma_start(out=dl_t[:], in_=dstloc[:])
            wt_t = cpool.tile([D, D], f32)
            nc.sync.dma_start(out=wt_t[:], in_=wt[:])
            iota_t = cpool.tile([P, P], f32)
            nc.sync.dma_start(out=iota_t[:], in_=iota_in[:])
            b_t = cpool.tile([1, D], f32)
            nc.sync.dma_start(out=b_t[:], in_=brow[:])
            ones_t = cpool.tile([1, P], f32)
            nc.vector.memset(ones_t[:], 1.0)

            for t in range(TILES_PER_CORE):
                ma_t = ma_pool.tile([P, cap_a, D], f32, tag="ma")
                for g0 in range(0, cap_a, G):
                    gc = min(G, cap_a - g0)
                    na = gc * P
                    o0 = (t * cap_a + g0) * P // 16
                    nc.gpsimd.dma_gather(
                        out_ap=ma_t[:, g0 : g0 + gc, :],
                        in_ap=x[:],
                        idxs_ap=idxa_t[:, o0 : o0 + na // 16],
                        num_idxs=na,
                        num_idxs_reg=na,
                        elem_size=D,
                        elem_step=D,
                    )
                mb_t = mb_pool.tile([P, cap_b, D], f32, tag="mb")
                for g0 in range(0, cap_b, G):
                    gc = min(G, cap_b - g0)
                    nb = gc * P
                    o0 = (t * cap_b + g0) * P // 16
                    nc.gpsimd.dma_gather(
                        out_ap=mb_t[:, g0 : g0 + gc, :],
                        in_ap=xb[:],
                        idxs_ap=idxb_t[:, o0 : o0 + nb // 16],
                        num_idxs=nb,
                        num_idxs_reg=nb,
                        elem_size=D,
                        elem_step=D,
                    )
                ps_ht = ps1_pool.tile([P, P], f32, tag="psht")
                for c in range(cap):
                    pt = p_pool.tile([P, P], f32, tag="pt")
                    nc.vector.tensor_scalar(
                        out=pt[:],
                        in0=iota_t[:],
                        scalar1=dl_t[:, t * cap + c : t * cap + c + 1],
                        scalar2=None,
                        op0=mybir.AluOpType.is_equal,
                    )
                    lhs = ma_t[:, c, :] if c < cap_a else mb_t[:, c - cap_a, :]
                    nc.tensor.matmul(
                        out=ps_ht[:],
                        lhsT=lhs,
                        rhs=pt[:],
                        start=(c == 0),
                        stop=(c == cap - 1),
                    )
                ht_t = ht_pool.tile([P, P], f32, tag="ht")
                nc.scalar.copy(out=ht_t[:], in_=ps_ht[:])
                ps_o = ps2_pool.tile([P, D], f32, tag="pso")
                nc.tensor.matmul(
                    out=ps_o[:], lhsT=ht_t[:], rhs=wt_t[:], start=True, stop=False
                )
                nc.tensor.matmul(
                    out=ps_o[:], lhsT=ones_t[:], rhs=b_t[:], start=False, stop=True
                )
                o_t = o_pool.tile([P, D], f32, tag="ot")
                nc.scalar.copy(out=o_t[:], in_=ps_o[:])
                nc.sync.dma_start(out=out[t * P : (t + 1) * P, :], in_=o_t[:])

    nc.finalize()
    return nc


def get_program(cap_a: int, cap_b: int):
    key = (cap_a, cap_b)
    if key not in _PROGRAM_CACHE:
        _PROGRAM_CACHE[key] = _build_program(cap_a, cap_b)
    return _PROGRAM_CACHE[key]


def kernel(x, src, dst, W, b):
    from concourse.bass_utils import run_bass_kernel_spmd

    in_maps, (cap_a, cap_b), slot_node = _prepare(x, src, dst, W, b)
    nc = get_program(cap_a, cap_b)
    res = run_bass_kernel_spmd(nc, in_maps, list(range(N_CORES)))

    full = np.empty((N_NODES, D), dtype=np.float32)
    for c in range(N_CORES):
        o = res.results[c]["out"]
        sn = slot_node[c * SLOTS_PER_CORE : (c + 1) * SLOTS_PER_CORE]
        valid = sn >= 0
        full[sn[valid]] = o[valid]
    return full



# revision 7
# speedup vs baseline: 1.2174x; 1.2174x over previous
"""GCN message-passing layer on 8 Trainium2 NeuronCores.

out = segment_sum(x[src], dst, N) @ W.T + b

Strategy (per core, dst-sharded):
  - Host: greedily bin-pack dst nodes into 320 bins (<=128 nodes each),
    balancing in-degree; bins are dealt to (core, tile) slots sorted by
    A-half edge count so every core's tile t has a near-identical chunk
    count (the SPMD program is shared across cores, so per-tile chunk
    capacity is the max over cores -- aligning counts keeps padding tight,
    and makes per-tile sizes descending, which shrinks the pipeline tail).
    Edges are grouped per (tile, src-half) into 128-edge chunks (int16
    gather indices force splitting the 40000-row x table at row 32768).
    x is pre-cast to bf16 on host.
  - Device: the whole bf16 message stream lives in two SBUF arenas
    (~165 KB/partition); dma_gather (SWDGE) calls of 8 chunks (1024
    indices -- the SWDGE ring limit) stream x rows in, crossing tile
    boundaries, A/B interleaved by ready-tile. A one-hot matrix P (DVE
    iota==dst_lane, bf16) scatters each 128-edge chunk into a PSUM
    accumulator via PE matmul (bf16, 1 cyc/row); a second PE matmul
    applies W.T and the bias.
  - Host: inverse-permute the per-core outputs back to node order.
"""

import sys

import numpy as np

sys.path.insert(0, "/opt/trn_rl_repo")

N_NODES = 40000
N_EDGES = 640000
D = 128
P = 128
N_CORES = 8
TILES_PER_CORE = 40
N_BINS = N_CORES * TILES_PER_CORE  # 320 tiles of up to 128 nodes
SLOTS_PER_CORE = TILES_PER_CORE * P  # 5120
HALF = 32768  # int16 gather index limit
PAD_DST = 1000.0  # dst_local sentinel for pad slots -> all-zero one-hot column
CALL_CHUNKS = 8  # 1024 indices per dma_gather call -- hard SWDGE ring limit

_PROGRAM_CACHE: dict = {}


def _pack_nodes(dst: np.ndarray):
    """Greedy LPT pack of nodes into N_BINS bins (<=128 nodes each),
    balancing per-bin in-degree. Returns bin/lane maps."""
    import heapq

    deg = np.bincount(dst, minlength=N_NODES)
    order = np.argsort(-deg, kind="stable")

    heap = [(0, b) for b in range(N_BINS)]
    heapq.heapify(heap)
    counts = np.zeros(N_BINS, dtype=np.int64)
    node_bin = np.empty(N_NODES, dtype=np.int32)
    node_lane = np.empty(N_NODES, dtype=np.int32)
    for n in order:
        while True:
            s, b = heapq.heappop(heap)
            if counts[b] < P:
                break
        node_bin[n] = b
        node_lane[n] = counts[b]
        counts[b] += 1
        heapq.heappush(heap, (s + int(deg[n]), b))
    return node_bin, node_lane


def _wrap_idx(arr: np.ndarray) -> np.ndarray:
    """int16 flat idx list -> [128, len/16] wrapped + replicated layout."""
    w = arr.reshape(-1, 16).T  # [16, n/16]
    return np.ascontiguousarray(np.tile(w, (8, 1)))


def _prepare(x, src, dst, W, b):
    import ml_dtypes

    src = np.asarray(src).astype(np.int64)
    dst = np.asarray(dst).astype(np.int64)
    node_bin, node_lane = _pack_nodes(dst)

    # Deal bins (sorted by A-half count) to (core, tile) slots: sorted rank
    # i -> core i%8, tile i//8. Aligns per-tile counts across cores and makes
    # tile sizes descending within each core.
    ehalf = (src >= HALF).astype(np.int64)
    binA = np.bincount(node_bin[dst[ehalf == 0]], minlength=N_BINS)
    rank = np.argsort(-binA, kind="stable")
    bin_core = np.empty(N_BINS, dtype=np.int64)
    bin_tile = np.empty(N_BINS, dtype=np.int64)
    bin_core[rank] = np.arange(N_BINS) % N_CORES
    bin_tile[rank] = np.arange(N_BINS) // N_CORES

    ebin = node_bin[dst]
    etile = bin_core[ebin] * TILES_PER_CORE + bin_tile[ebin]
    edl = node_lane[dst].astype(np.float32)
    eorder = np.lexsort((src, ehalf, etile))
    s_src, s_dl = src[eorder], edl[eorder]
    key = etile[eorder] * 2 + ehalf[eorder]

    seg_start = np.searchsorted(key, np.arange(N_BINS * 2), side="left")
    seg_end = np.searchsorted(key, np.arange(N_BINS * 2), side="right")
    cntA = (seg_end[0::2] - seg_start[0::2]).reshape(N_CORES, TILES_PER_CORE)
    cntB = (seg_end[1::2] - seg_start[1::2]).reshape(N_CORES, TILES_PER_CORE)
    maxA = cntA.max(axis=0)
    maxB = np.maximum(1, cntB.max(axis=0))
    capA = -(-maxA // P)  # chunks per tile
    capB = -(-maxB // P)
    aoff = np.concatenate(([0], np.cumsum(capA)))
    boff = np.concatenate(([0], np.cumsum(capB)))
    choff = np.concatenate(([0], np.cumsum(capA + capB)))
    nchunk = int(aoff[-1] + boff[-1])

    x_bf = np.asarray(x).astype(ml_dtypes.bfloat16)
    xa = np.ascontiguousarray(x_bf[:HALF])
    xb = np.ascontiguousarray(x_bf[HALF:])
    iota = np.tile(
        np.arange(P, dtype=np.float32)[None, :], (P, 1)
    ).astype(ml_dtypes.bfloat16)
    wt = np.ascontiguousarray(np.asarray(W).T.astype(ml_dtypes.bfloat16))
    brow = np.asarray(b).astype(ml_dtypes.bfloat16)[None, :]

    in_maps = []
    for c in range(N_CORES):
        idxA = np.zeros(int(aoff[-1]) * P, dtype=np.int16)
        idxB = np.zeros(int(boff[-1]) * P, dtype=np.int16)
        dl = np.full((nchunk, P), PAD_DST, dtype=np.float32)
        for t in range(TILES_PER_CORE):
            g = (c * TILES_PER_CORE + t) * 2
            a0, a1 = seg_start[g], seg_end[g]
            nA = a1 - a0
            idxA[aoff[t] * P : aoff[t] * P + nA] = s_src[a0:a1].astype(np.int16)
            dlA = dl[choff[t] : choff[t] + capA[t]].reshape(-1)
            dlA[:nA] = s_dl[a0:a1]
            b0, b1 = seg_start[g + 1], seg_end[g + 1]
            nB = b1 - b0
            idxB[boff[t] * P : boff[t] * P + nB] = (
                s_src[b0:b1] - HALF
            ).astype(np.int16)
            dlB = dl[choff[t] + capA[t] : choff[t] + capA[t] + capB[t]].reshape(-1)
            dlB[:nB] = s_dl[b0:b1]

        in_maps.append(
            {
                "x": xa,
                "xb": xb,
                "idxa": _wrap_idx(idxA),
                "idxb": _wrap_idx(idxB),
                "dstloc": np.ascontiguousarray(dl.T),  # [128 lanes, nchunk]
                "wt": wt,
                "iota": iota,
                "brow": brow,
            }
        )

    slot = (bin_core[node_bin] * TILES_PER_CORE + bin_tile[node_bin]) * P + node_lane
    slot_node = np.full(N_BINS * P, -1, dtype=np.int64)
    slot_node[slot] = np.arange(N_NODES)
    caps = (tuple(int(v) for v in capA), tuple(int(v) for v in capB))
    return in_maps, caps, slot_node


def _build_program(capA: tuple, capB: tuple):
    import concourse.mybir as mybir
    import concourse.tile as tile
    from concourse import bacc

    f32 = mybir.dt.float32
    bf16 = mybir.dt.bfloat16
    i16 = mybir.dt.int16

    capA = np.array(capA)
    capB = np.array(capB)
    aoff = np.concatenate(([0], np.cumsum(capA)))
    boff = np.concatenate(([0], np.cumsum(capB)))
    choff = np.concatenate(([0], np.cumsum(capA + capB)))
    nchunk = int(aoff[-1] + boff[-1])
    CA, CB = int(aoff[-1]), int(boff[-1])

    nc = bacc.Bacc("TRN2")
    x = nc.dram_tensor("x", [HALF, D], bf16, kind="ExternalInput")
    xb = nc.dram_tensor("xb", [N_NODES - HALF, D], bf16, kind="ExternalInput")
    idxa = nc.dram_tensor("idxa", [P, CA * P // 16], i16, kind="ExternalInput")
    idxb = nc.dram_tensor("idxb", [P, CB * P // 16], i16, kind="ExternalInput")
    dstloc = nc.dram_tensor("dstloc", [P, nchunk], f32, kind="ExternalInput")
    wt = nc.dram_tensor("wt", [D, D], bf16, kind="ExternalInput")
    iota_in = nc.dram_tensor("iota", [P, P], bf16, kind="ExternalInput")
    brow = nc.dram_tensor("brow", [1, D], bf16, kind="ExternalInput")
    out = nc.dram_tensor("out", [SLOTS_PER_CORE, D], f32, kind="ExternalOutput")

    # Gather calls: CALL_CHUNKS-chunk groups of each stream, crossing tile
    # boundaries. Interleave A and B calls by the last tile each call
    # completes, so tile processing can chase the stream.
    tile_of_a = np.searchsorted(aoff[1:], np.arange(CA), side="right")
    tile_of_b = np.searchsorted(boff[1:], np.arange(CB), side="right")
    calls = []
    for s in range(0, CA, CALL_CHUNKS):
        e = min(s + CALL_CHUNKS, CA)
        calls.append((int(tile_of_a[e - 1]), 0, s, e))
    for s in range(0, CB, CALL_CHUNKS):
        e = min(s + CALL_CHUNKS, CB)
        calls.append((int(tile_of_b[e - 1]), 1, s, e))
    calls.sort()

    with tile.TileContext(nc) as tc:
        with (
            tc.tile_pool(name="const", bufs=1) as cpool,
            tc.tile_pool(name="pt", bufs=6) as p_pool,
            tc.tile_pool(name="ht", bufs=3) as ht_pool,
            tc.tile_pool(name="ot", bufs=3) as o_pool,
            tc.tile_pool(name="ps1", bufs=4, space="PSUM") as ps1_pool,
            tc.tile_pool(name="ps2", bufs=2, space="PSUM") as ps2_pool,
        ):
            # idx tables first -- they gate the first gathers.
            idxa_t = cpool.tile([P, CA * P // 16], i16)
            nc.sync.dma_start(out=idxa_t[:], in_=idxa[:])
            idxb_t = cpool.tile([P, CB * P // 16], i16)
            nc.sync.dma_start(out=idxb_t[:], in_=idxb[:])
            dl_t = cpool.tile([P, nchunk], f32)
            nc.sync.dma_start(out=dl_t[:], in_=dstloc[:])
            iota_t = cpool.tile([P, P], bf16)
            nc.sync.dma_start(out=iota_t[:], in_=iota_in[:])
            wt_t = cpool.tile([D, D], bf16)
            nc.sync.dma_start(out=wt_t[:], in_=wt[:])
            b_t = cpool.tile([1, D], bf16)
            nc.sync.dma_start(out=b_t[:], in_=brow[:])
            ones_t = cpool.tile([1, P], bf16)
            nc.vector.memset(ones_t[:], 1.0)

            # whole-stream SBUF arenas; each chunk written exactly once
            arena_a = cpool.tile([P, CA, D], bf16)
            arena_b = cpool.tile([P, CB, D], bf16)
            for _, stream, s, e in calls:
                n = (e - s) * P
                if stream == 0:
                    nc.gpsimd.dma_gather(
                        out_ap=arena_a[:, s:e, :],
                        in_ap=x[:],
                        idxs_ap=idxa_t[:, s * P // 16 : e * P // 16],
                        num_idxs=n,
                        num_idxs_reg=n,
                        elem_size=D,
                        elem_step=D,
                    )
                else:
                    nc.gpsimd.dma_gather(
                        out_ap=arena_b[:, s:e, :],
                        in_ap=xb[:],
                        idxs_ap=idxb_t[:, s * P // 16 : e * P // 16],
                        num_idxs=n,
                        num_idxs_reg=n,
                        elem_size=D,
                        elem_step=D,
                    )

            for t in range(TILES_PER_CORE):
                nA_c, nB_c = int(capA[t]), int(capB[t])
                cap_t = nA_c + nB_c
                ps_ht = ps1_pool.tile([P, P], f32, tag="psht")
                for cci in range(cap_t):
                    col = int(choff[t]) + cci
                    pt = p_pool.tile([P, P], bf16, tag="pt")
                    nc.vector.tensor_scalar(
                        out=pt[:],
                        in0=iota_t[:],
                        scalar1=dl_t[:, col : col + 1],
                        scalar2=None,
                        op0=mybir.AluOpType.is_equal,
                    )
                    if cci < nA_c:
                        lhs = arena_a[:, int(aoff[t]) + cci, :]
                    else:
                        lhs = arena_b[:, int(boff[t]) + cci - nA_c, :]
                    nc.tensor.matmul(
                        out=ps_ht[:],
                        lhsT=lhs,
                        rhs=pt[:],
                        start=(cci == 0),
                        stop=(cci == cap_t - 1),
                    )
                ht_t = ht_pool.tile([P, P], bf16, tag="ht")
                nc.scalar.copy(out=ht_t[:], in_=ps_ht[:])
                ps_o = ps2_pool.tile([P, D], f32, tag="pso")
                nc.tensor.matmul(
                    out=ps_o[:], lhsT=ht_t[:], rhs=wt_t[:], start=True, stop=False
                )
                nc.tensor.matmul(
                    out=ps_o[:], lhsT=ones_t[:], rhs=b_t[:], start=False, stop=True
                )
                o_t = o_pool.tile([P, D], f32, tag="ot")
                nc.scalar.copy(out=o_t[:], in_=ps_o[:])
                nc.sync.dma_start(out=out[t * P : (t + 1) * P, :], in_=o_t[:])

    nc.finalize()
    return nc


def get_program(capA, capB):
    key = (tuple(capA), tuple(capB))
    if key not in _PROGRAM_CACHE:
        _PROGRAM_CACHE[key] = _build_program(*key)
    return _PROGRAM_CACHE[key]


def kernel(x, src, dst, W, b):
    from concourse.bass_utils import run_bass_kernel_spmd

    in_maps, caps, slot_node = _prepare(x, src, dst, W, b)
    nc = get_program(*caps)
    res = run_bass_kernel_spmd(nc, in_maps, list(range(N_CORES)))

    full = np.empty((N_NODES, D), dtype=np.float32)
    for c in range(N_CORES):
        o = res.results[c]["out"]
        sn = slot_node[c * SLOTS_PER_CORE : (c + 1) * SLOTS_PER_CORE]
        valid = sn >= 0
        full[sn[valid]] = o[valid]
    return full
